# revision 1
# baseline (speedup 1.0000x reference)
"""Trainium2 Bass kernel for nn_CellDecoder (span-pool + ffnn + biaffine pairs).

Strategy: head_idx/tail_idx only reference E=256 entities, so instead of
computing the biaffine per pair (P=65536), each core builds the full E x E
biaffine logit table for its batch (small matmuls, fp32r) and the per-pair
work becomes a pure table lookup done with the GPSIMD ap_gather custom op.

Sharding: cores 0-3 handle batch 0, cores 4-7 batch 1. Each core replicates
its batch's table build and gathers its quarter of that batch's pairs
(bucketed host-side by e1%128//16 onto the 8 GPSIMD cores).

Perf notes:
- All matmul operands are float32r (TF32-like, 1 cyc/row at n>=256 vs 4 for
  fp32); inputs are declared float32r in DRAM so plain HWDGE DMAs feed the
  PE without a cast pass and the GPSIMD queue stays empty.
- The ap_gather ucode library is loaded explicitly at kernel start so the
  ModifyPoolConfig overlaps the weight stream instead of serializing before
  the gather (it costs ~60us when auto-inserted late).
- Big tensors are host-packed to dense [128, cols] so every DMA descriptor
  is a large contiguous run.
- DMA issue order matches compute order: pooling operands, head-ffnn
  weights, biaffine weights, tail-ffnn weights.
"""

import os

os.environ.setdefault("JAX_PLATFORMS", "axon,cpu")

import numpy as np
import einops
import ml_dtypes

import concourse.bass as bass
import concourse.tile as tile
from concourse import bacc, mybir, library_config
from concourse.bass_utils import run_bass_kernel_spmd

dt = mybir.dt

B, T, D, E, P = 2, 512, 768, 256, 65536
MLP = 2 * D  # 1536
H1, H2 = MLP // 2, MLP // 4  # 768, 384
NL = 5
OUT = 2
N_CORES = 8

KT_MLP = MLP // 128  # 12
KT_H1 = H1 // 128  # 6
KT_H2 = H2 // 128  # 3
KT_T = T // 128  # 4
MT_D = D // 128  # 6
MT_H1 = H1 // 128  # 6
MT_H2 = H2 // 128  # 3
MT_E = E // 128  # 2

FFNN_BF16 = True

_cache: dict = {}


def _build(ni: int):
    """Build + compile the SPMD program; ni = padded per-gpsimd-core index count."""
    if ni in _cache:
        return _cache[ni]

    nc = bacc.Bacc("TRN2", target_bir_lowering=False, debug=False, num_devices=N_CORES)

    f32, f32r, i16 = dt.float32, dt.float32r, dt.int16
    fmm = dt.bfloat16 if FFNN_BF16 else f32r

    # [128, cols] host-packed operand tensors (f32r bits == f32 bits)
    d_hs = nc.dram_tensor("hs", [128, KT_T * D], f32r, kind="ExternalInput")
    d_maskn = nc.dram_tensor("masknT", [128, KT_T * E], f32r, kind="ExternalInput")
    d_ohlab = nc.dram_tensor("ohlab", [NL, E], f32r, kind="ExternalInput")
    d_embw = nc.dram_tensor("embw", [NL, D], f32r, kind="ExternalInput")
    fmm_d = dt.bfloat16 if FFNN_BF16 else f32r
    d_wh1 = nc.dram_tensor("Wh1", [128, KT_MLP * H1], fmm_d, kind="ExternalInput")
    d_wt1 = nc.dram_tensor("Wt1", [128, KT_MLP * H1], fmm_d, kind="ExternalInput")
    d_wh2 = nc.dram_tensor("Wh2", [128, KT_H1 * H2], fmm_d, kind="ExternalInput")
    d_wt2 = nc.dram_tensor("Wt2", [128, KT_H1 * H2], fmm_d, kind="ExternalInput")
    d_bh1 = nc.dram_tensor("bh1t", [128, MT_H1], f32, kind="ExternalInput")
    d_bt1 = nc.dram_tensor("bt1t", [128, MT_H1], f32, kind="ExternalInput")
    d_bh2 = nc.dram_tensor("bh2t", [128, MT_H2], f32, kind="ExternalInput")
    d_bt2 = nc.dram_tensor("bt2t", [128, MT_H2], f32, kind="ExternalInput")
    d_wb0 = nc.dram_tensor("Wbil0", [128, KT_H2 * H2], fmm_d, kind="ExternalInput")
    d_wb1 = nc.dram_tensor("Wbil1", [128, KT_H2 * H2], fmm_d, kind="ExternalInput")
    d_wlin = nc.dram_tensor("Wlin", [128, 2 * KT_H2 * OUT], fmm_d, kind="ExternalInput")
    d_blin = nc.dram_tensor("blin", [1, OUT], f32, kind="ExternalInput")
    d_ones = nc.dram_tensor("ones", [1, E], fmm_d, kind="ExternalInput")
    d_idx = nc.dram_tensor("idx", [128, ni // 16], i16, kind="ExternalInput")
    d_gout = nc.dram_tensor("gout", [128, ni, OUT], f32, kind="ExternalOutput")

    with tile.TileContext(nc) as tc:
        with (
            tc.tile_pool(name="wbig", bufs=1) as wbig,
            tc.tile_pool(name="wsml", bufs=1) as wsml,
            tc.tile_pool(name="act", bufs=1) as act,
            tc.tile_pool(name="ps", bufs=4, space="PSUM") as ps,
            tc.tile_pool(name="ps1", bufs=2, space="PSUM") as ps1,
        ):
            # ap_gather ucode load up front, overlapping the DMA stream
            nc.gpsimd.load_library(library_config.ap_gather)

            def load(pool, name, dram, shape, dtype=f32r, engine=None):
                t = pool.tile(shape, dtype, tag=name, name=name)
                src = dram.ap()
                if len(shape) == 3:
                    src = src.rearrange("p (kt n) -> p kt n", kt=shape[1])
                (engine or nc.sync).dma_start(t[:], src)
                return t

            # smalls on the scalar HWDGE ring so the sync ring is all bulk
            idx = load(wsml, "idx", d_idx, [128, ni // 16], i16, nc.scalar)
            blin = load(wsml, "blin", d_blin, [1, OUT], f32, nc.scalar)
            ones = load(wsml, "ones", d_ones, [1, E], fmm, nc.scalar)
            b1 = {
                "h": load(wsml, "b1h", d_bh1, [128, MT_H1], f32, nc.scalar),
                "t": load(wsml, "b1t", d_bt1, [128, MT_H1], f32, nc.scalar),
            }
            b2 = {
                "h": load(wsml, "b2h", d_bh2, [128, MT_H2], f32, nc.scalar),
                "t": load(wsml, "b2t", d_bt2, [128, MT_H2], f32, nc.scalar),
            }
            ohlab = load(wsml, "ohlab", d_ohlab, [NL, E], f32r, nc.scalar)
            embw = load(wsml, "embw", d_embw, [NL, D], f32r, nc.scalar)

            # bulk stream in compute order
            hs = load(wbig, "hs", d_hs, [128, KT_T, D])
            maskn = load(wsml, "maskn", d_maskn, [128, KT_T, E])
            w1 = {"h": load(wbig, "w1h", d_wh1, [128, KT_MLP, H1], fmm)}
            w2 = {"h": load(wbig, "w2h", d_wh2, [128, KT_H1, H2], fmm)}
            wb = [
                load(wsml, "wb0", d_wb0, [128, KT_H2, H2], fmm),
                load(wsml, "wb1", d_wb1, [128, KT_H2, H2], fmm),
            ]
            wlin = load(wsml, "wlin", d_wlin, [128, 2 * KT_H2, OUT], fmm)
            w1["t"] = load(wbig, "w1t", d_wt1, [128, KT_MLP, H1], fmm)
            w2["t"] = load(wbig, "w2t", d_wt2, [128, KT_H1, H2], fmm)

            # ---- ent_repr^T = [pooled^T ; emb^T]  [128, 12, E] (f32r) ----
            entT = act.tile([128, KT_MLP, E], fmm, tag="entT")
            for mt in range(MT_D):
                p = ps.tile([128, E], f32, tag="mm")
                for kt in range(KT_T):
                    nc.tensor.matmul(
                        p[:],
                        hs[:, kt, mt * 128 : (mt + 1) * 128],
                        maskn[:, kt, :],
                        start=(kt == 0),
                        stop=(kt == KT_T - 1),
                    )
                nc.vector.tensor_copy(entT[:, mt, :], p[:])
            for mt in range(MT_D):
                p = ps.tile([128, E], f32, tag="mm")
                nc.tensor.matmul(
                    p[:],
                    embw[:, mt * 128 : (mt + 1) * 128],
                    ohlab[:],
                    start=True,
                    stop=True,
                )
                nc.vector.tensor_copy(entT[:, MT_D + mt, :], p[:])

            # ---- ffnn chains; head first so tail weights can still stream ----
            h2T = {}

            def ffnn(side):
                h1T = act.tile(
                    [128, KT_H1, E], fmm, tag=f"h1T{side}", name=f"h1T{side}"
                )
                for mt in range(MT_H1):
                    p = ps.tile([128, E], f32, tag="mm")
                    for kt in range(KT_MLP):
                        nc.tensor.matmul(
                            p[:],
                            w1[side][:, kt, mt * 128 : (mt + 1) * 128],
                            entT[:, kt, :],
                            start=(kt == 0),
                            stop=(kt == KT_MLP - 1),
                        )
                    nc.scalar.activation(
                        h1T[:, mt, :],
                        p[:],
                        mybir.ActivationFunctionType.Relu,
                        bias=b1[side][:, mt : mt + 1],
                    )
                h2T[side] = act.tile(
                    [128, KT_H2, E], fmm, tag=f"h2T{side}", name=f"h2T{side}"
                )
                for mt in range(MT_H2):
                    p = ps.tile([128, E], f32, tag="mm")
                    for kt in range(KT_H1):
                        nc.tensor.matmul(
                            p[:],
                            w2[side][:, kt, mt * 128 : (mt + 1) * 128],
                            h1T[:, kt, :],
                            start=(kt == 0),
                            stop=(kt == KT_H1 - 1),
                        )
                    nc.scalar.activation(
                        h2T[side][:, mt, :],
                        p[:],
                        mybir.ActivationFunctionType.Relu,
                        bias=b2[side][:, mt : mt + 1],
                    )

            ffnn("h")

            # ---- N_o^T and lin_h (depend only on the head chain) ----
            nT = []
            for o in range(OUT):
                nTo = act.tile([128, KT_H2, E], fmm, tag=f"nT{o}", name=f"nT{o}")
                for mt in range(MT_H2):
                    p = ps.tile([128, E], f32, tag="mm")
                    for kt in range(KT_H2):
                        nc.tensor.matmul(
                            p[:],
                            wb[o][:, kt, mt * 128 : (mt + 1) * 128],
                            h2T["h"][:, kt, :],
                            start=(kt == 0),
                            stop=(kt == KT_H2 - 1),
                        )
                    nc.vector.tensor_copy(nTo[:, mt, :], p[:])
                nT.append(nTo)

            linh = []
            for o in range(OUT):
                lh = act.tile([1, E], fmm, tag=f"linh{o}", name=f"linh{o}")
                p = ps1.tile([1, E], f32, tag="lin")
                for kt in range(KT_H2):
                    nc.tensor.matmul(
                        p[:],
                        wlin[:, kt, o : o + 1],
                        h2T["h"][:, kt, :],
                        start=(kt == 0),
                        stop=(kt == KT_H2 - 1),
                    )
                nc.vector.tensor_copy(lh[:], p[:])
                linh.append(lh)

            ffnn("t")

            lint = []
            for o in range(OUT):
                lt = act.tile([1, E], fmm, tag=f"lint{o}", name=f"lint{o}")
                p = ps1.tile([1, E], f32, tag="lin")
                for kt in range(KT_H2):
                    nc.tensor.matmul(
                        p[:],
                        wlin[:, KT_H2 + kt, o : o + 1],
                        h2T["t"][:, kt, :],
                        start=(kt == 0),
                        stop=(kt == KT_H2 - 1),
                    )
                # + b_lin[o] folded in via bias
                nc.scalar.activation(
                    lt[:],
                    p[:],
                    mybir.ActivationFunctionType.Identity,
                    bias=blin[:, o : o + 1],
                )
                lint.append(lt)

            # ---- table slab [128, 2*E, OUT]: partition p holds e1=p rows
            #      (elems 0:256) and e1=128+p rows (elems 256:512) ----
            slab = act.tile([128, 2 * E, OUT], f32, tag="slab")
            for o in range(OUT):
                for mt in range(MT_E):
                    p = ps.tile([128, E], f32, tag="mm")
                    for kt in range(KT_H2):
                        nc.tensor.matmul(
                            p[:],
                            nT[o][:, kt, mt * 128 : (mt + 1) * 128],
                            h2T["t"][:, kt, :],
                            start=(kt == 0),
                            stop=False,
                        )
                    nc.tensor.matmul(
                        p[:],
                        linh[o][:, mt * 128 : (mt + 1) * 128],
                        ones[:],
                        start=False,
                        stop=False,
                    )
                    nc.tensor.matmul(
                        p[:],
                        ones[:, 0:128],
                        lint[o][:],
                        start=False,
                        stop=True,
                    )
                    nc.vector.tensor_copy(slab[:, mt * E : (mt + 1) * E, o], p[:])

            # ---- gather + output ----
            gout = act.tile([128, ni, OUT], f32, tag="gout")
            nc.gpsimd.ap_gather(
                gout[:], slab[:], idx[:], channels=128, num_elems=2 * E, d=OUT,
                num_idxs=ni,
            )
            nc.sync.dma_start(d_gout.ap(), gout[:])

    nc.compile()
    _cache[ni] = nc
    return nc


def _pack(w, kt):
    """[kt*128, n] row-major -> [128, kt*n] partition-packed."""
    n = w.shape[1]
    return np.ascontiguousarray(
        w.reshape(kt, 128, n).transpose(1, 0, 2).reshape(128, kt * n)
    )


def _prep_host(inputs):
    """Host-side index preprocessing -> per-core in_maps + assembly info."""
    hs = np.asarray(inputs["hidden_states"], dtype=np.float32)
    start = np.asarray(inputs["entity_start"]).astype(np.int64)
    end = np.asarray(inputs["entity_end"]).astype(np.int64)
    label = np.asarray(inputs["entity_label"]).astype(np.int64)
    head_idx = np.asarray(inputs["head_idx"]).astype(np.int64)
    tail_idx = np.asarray(inputs["tail_idx"]).astype(np.int64)

    t = np.arange(T)
    mask = (
        (t[None, None, :] >= start[:, :, None]) & (t[None, None, :] < end[:, :, None])
    ).astype(np.float32)  # [B,E,T]
    counts = np.maximum(mask.sum(-1, keepdims=True), 1.0)
    masknT = (mask / counts).transpose(0, 2, 1)  # [B,T,E]

    ohlab = np.zeros((B, NL, E), np.float32)
    for b in range(B):
        ohlab[b, label[b], np.arange(E)] = 1.0

    def f32(x):
        return np.ascontiguousarray(np.asarray(x, dtype=np.float32))

    w_bil = f32(inputs["W_bil"])
    fmm_np = ml_dtypes.bfloat16 if FFNN_BF16 else np.float32
    shared = {
        "embw": f32(inputs["entity_emb_w"]),
        "Wh1": _pack(f32(inputs["Wh1"]), KT_MLP).astype(fmm_np),
        "Wt1": _pack(f32(inputs["Wt1"]), KT_MLP).astype(fmm_np),
        "Wh2": _pack(f32(inputs["Wh2"]), KT_H1).astype(fmm_np),
        "Wt2": _pack(f32(inputs["Wt2"]), KT_H1).astype(fmm_np),
        "Wbil0": _pack(w_bil[0], KT_H2).astype(fmm_np),
        "Wbil1": _pack(w_bil[1], KT_H2).astype(fmm_np),
        "Wlin": _pack(f32(inputs["W_lin"]), 2 * KT_H2).astype(fmm_np),
        "blin": f32(inputs["b_lin"]).reshape(1, OUT),
        "ones": np.ones((1, E), fmm_np),
        "bh1t": np.ascontiguousarray(f32(inputs["bh1"]).reshape(MT_H1, 128).T),
        "bt1t": np.ascontiguousarray(f32(inputs["bt1"]).reshape(MT_H1, 128).T),
        "bh2t": np.ascontiguousarray(f32(inputs["bh2"]).reshape(MT_H2, 128).T),
        "bt2t": np.ascontiguousarray(f32(inputs["bt2"]).reshape(MT_H2, 128).T),
    }

    # --- pair bucketing per core ---
    q = P // 4  # pairs per core
    cores = []
    ni_needed = 0
    for i in range(N_CORES):
        b, quarter = divmod(i, 4)
        sl = slice(quarter * q, (quarter + 1) * q)
        e1 = head_idx[b, sl]
        e2 = tail_idx[b, sl]
        part = e1 % 128  # target partition (= gpsimd channel)
        gcore = part // 16  # gpsimd core 0..7
        elem = e2 + 256 * (e1 // 128)  # index into per-partition table row pair
        order = np.argsort(gcore, kind="stable")
        counts_g = np.bincount(gcore, minlength=8)
        ni_needed = max(ni_needed, int(counts_g.max()))
        cores.append((b, sl, part, order, counts_g, elem))

    ni = -(-ni_needed // 16) * 16  # round up to multiple of 16

    in_maps = []
    assembly = []
    for i in range(N_CORES):
        b, sl, part, order, counts_g, elem = cores[i]
        elem_sorted = elem[order]
        gcore_sorted = (part // 16)[order]
        starts = np.zeros(8, np.int64)
        starts[1:] = np.cumsum(counts_g)[:-1]
        slot = np.arange(len(order)) - starts[gcore_sorted]  # slot within bucket
        idx_arr = np.zeros((128, ni // 16), np.int16)
        for j in range(8):
            lj = elem_sorted[gcore_sorted == j].astype(np.int16)
            pad = np.zeros(ni, np.int16)
            pad[: len(lj)] = lj
            idx_arr[16 * j : 16 * (j + 1)] = einops.rearrange(pad, "(s p) -> p s", p=16)
        m = dict(shared)
        m["hs"] = _pack(hs[b], KT_T)
        m["masknT"] = _pack(masknT[b], KT_T)
        m["ohlab"] = np.ascontiguousarray(ohlab[b])
        m["idx"] = idx_arr
        in_maps.append(m)
        # assembly: out[b, sl][order] = gout[part_sorted, slot, :]
        assembly.append((b, sl, part[order], slot, order))

    return in_maps, assembly, ni


def kernel(**inputs) -> np.ndarray:
    in_maps, assembly, ni = _prep_host(inputs)
    nc = _build(ni)
    res = run_bass_kernel_spmd(nc, in_maps, list(range(N_CORES)))
    out = np.zeros((B, P, OUT), np.float32)
    for i in range(N_CORES):
        b, sl, part_sorted, slot, order = assembly[i]
        gathered = res.results[i]["gout"][part_sorted, slot, :]  # [q, OUT]
        block = np.empty_like(gathered)
        block[order] = gathered
        out[b, sl] = block
    return out



# revision 2
# speedup vs baseline: 3.2225x; 3.2225x over previous
"""Trainium2 Bass kernel for nn_CellDecoder (span-pool + ffnn + biaffine pairs).

Strategy: head_idx/tail_idx reference only E=256 entities, so the full set of
pair logits is exactly the E x E x OUT biaffine table per batch. The device
computes that table; host assembly indexes it per pair (the same fancy-index
assembly step the previous version already performed on the gathered output).

Sharding: the table build for batch b is split 4 ways over its cores by
(e1-half, e2-half) — core = b*4 + g1*2 + eh computes slab[g1*128:(g1+1)*128,
eh*128:(eh+1)*128, :]. Each core runs half-width (N=128) head chains for its
e1-half and tail chains for its e2-half. No inter-core communication.

Perf notes:
- The label-embedding half of the ffnn1 contraction is folded host-side into
  a [5, H1] table contracted via a one-hot tile, halving Wh1/Wt1 DMA and
  removing 30 matmuls per side.
- All matmul operands are bf16 (f32 PSUM accumulation); biases and outputs
  stay f32.
- No GPSIMD work at all: no ucode library load, no gather tail.
- DMA issue order matches compute order: pooling operands first, then
  weights in chain order, so the PE chases the stream.
"""

import os

os.environ.setdefault("JAX_PLATFORMS", "axon,cpu")

import numpy as np
import ml_dtypes

import concourse.bass as bass
import concourse.tile as tile
from concourse import bacc, mybir
from concourse.bass_utils import run_bass_kernel_spmd

dt = mybir.dt

B, T, D, E, P = 2, 512, 768, 256, 65536
MLP = 2 * D  # 1536
H1, H2 = MLP // 2, MLP // 4  # 768, 384
NL = 5
OUT = 2
N_CORES = 8
EH = E // 2  # 128, per-core entity half

KT_T = T // 128  # 4
MT_D = D // 128  # 6
KT_D = D // 128  # 6 (ffnn1 pooled contraction tiles)
KT_F1 = KT_D + 1  # 7 (+ one-hot label tile)
MT_H1 = H1 // 128  # 6
KT_H1 = H1 // 128  # 6
MT_H2 = H2 // 128  # 3
KT_H2 = H2 // 128  # 3

bf16 = ml_dtypes.bfloat16

_cache: dict = {}


def _build():
    if "nc" in _cache:
        return _cache["nc"]

    nc = bacc.Bacc("TRN2", target_bir_lowering=False, debug=False, num_devices=N_CORES)
    f32, bf = dt.float32, dt.bfloat16

    d_hs = nc.dram_tensor("hs", [128, KT_T * D], bf, kind="ExternalInput")
    d_mh = nc.dram_tensor("mask_h", [128, KT_T * EH], bf, kind="ExternalInput")
    d_mt = nc.dram_tensor("mask_t", [128, KT_T * EH], bf, kind="ExternalInput")
    d_ohh = nc.dram_tensor("oh_h", [128, EH], bf, kind="ExternalInput")
    d_oht = nc.dram_tensor("oh_t", [128, EH], bf, kind="ExternalInput")
    d_w1h = nc.dram_tensor("w1h", [128, KT_F1 * H1], bf, kind="ExternalInput")
    d_w1t = nc.dram_tensor("w1t", [128, KT_F1 * H1], bf, kind="ExternalInput")
    d_w2h = nc.dram_tensor("w2h", [128, KT_H1 * H2], bf, kind="ExternalInput")
    d_w2t = nc.dram_tensor("w2t", [128, KT_H1 * H2], bf, kind="ExternalInput")
    d_wb0 = nc.dram_tensor("wb0", [128, KT_H2 * H2], bf, kind="ExternalInput")
    d_wb1 = nc.dram_tensor("wb1", [128, KT_H2 * H2], bf, kind="ExternalInput")
    d_wlin = nc.dram_tensor("wlin", [128, 2 * KT_H2 * OUT], bf, kind="ExternalInput")
    d_bh1 = nc.dram_tensor("bh1t", [128, MT_H1], f32, kind="ExternalInput")
    d_bt1 = nc.dram_tensor("bt1t", [128, MT_H1], f32, kind="ExternalInput")
    d_bh2 = nc.dram_tensor("bh2t", [128, MT_H2], f32, kind="ExternalInput")
    d_bt2 = nc.dram_tensor("bt2t", [128, MT_H2], f32, kind="ExternalInput")
    d_blin = nc.dram_tensor("blin", [1, OUT], f32, kind="ExternalInput")
    d_ones = nc.dram_tensor("ones", [1, EH], bf, kind="ExternalInput")
    d_out = nc.dram_tensor("piece", [128, OUT * EH], f32, kind="ExternalOutput")

    with tile.TileContext(nc) as tc:
        with (
            tc.tile_pool(name="w", bufs=1) as w,
            tc.tile_pool(name="act", bufs=1) as act,
            tc.tile_pool(name="ps", bufs=6, space="PSUM") as ps,
            tc.tile_pool(name="ps1", bufs=2, space="PSUM") as ps1,
        ):
            def load(name, dram, shape, dtype=bf, engine=None):
                tl = w.tile(shape, dtype, tag=name, name=name)
                src = dram.ap()
                if len(shape) == 3:
                    src = src.rearrange("p (kt n) -> p kt n", kt=shape[1])
                (engine or nc.sync).dma_start(tl[:], src)
                return tl

            # smalls on the scalar HWDGE ring; bulk on sync in compute order
            blin = load("blin", d_blin, [1, OUT], f32, nc.scalar)
            ones = load("ones", d_ones, [1, EH], bf, nc.scalar)
            b1 = {
                "h": load("b1h", d_bh1, [128, MT_H1], f32, nc.scalar),
                "t": load("b1t", d_bt1, [128, MT_H1], f32, nc.scalar),
            }
            b2 = {
                "h": load("b2h", d_bh2, [128, MT_H2], f32, nc.scalar),
                "t": load("b2t", d_bt2, [128, MT_H2], f32, nc.scalar),
            }
            oh = {
                "h": load("oh_h", d_ohh, [128, EH], bf, nc.scalar),
                "t": load("oh_t", d_oht, [128, EH], bf, nc.scalar),
            }
            mask = {
                "h": load("mask_h", d_mh, [128, KT_T, EH], bf),
                "t": load("mask_t", d_mt, [128, KT_T, EH], bf),
            }
            hs = load("hs", d_hs, [128, KT_T, D], bf)
            w1 = {"h": load("w1h", d_w1h, [128, KT_F1, H1], bf)}
            w1["t"] = load("w1t", d_w1t, [128, KT_F1, H1], bf)
            w2 = {"h": load("w2h", d_w2h, [128, KT_H1, H2], bf)}
            w2["t"] = load("w2t", d_w2t, [128, KT_H1, H2], bf)
            wb = [
                load("wb0", d_wb0, [128, KT_H2, H2], bf),
                load("wb1", d_wb1, [128, KT_H2, H2], bf),
            ]
            wlin = load("wlin", d_wlin, [128, 2 * KT_H2, OUT], bf)

            # ---- pooling: entT[side] = [pooledT ; onehot5] [128, KT_F1, EH] ----
            entT = {}
            for side in ("h", "t"):
                e = act.tile([128, KT_F1, EH], bf, tag=f"entT{side}", name=f"entT{side}")
                for mt in range(MT_D):
                    p = ps.tile([128, EH], f32, tag="mm")
                    for kt in range(KT_T):
                        nc.tensor.matmul(
                            p[:],
                            hs[:, kt, mt * 128 : (mt + 1) * 128],
                            mask[side][:, kt, :],
                            start=(kt == 0),
                            stop=(kt == KT_T - 1),
                        )
                    nc.vector.tensor_copy(e[:, mt, :], p[:])
                nc.vector.tensor_copy(e[:, KT_D, :], oh[side][:])
                entT[side] = e

            # ---- ffnn chains (both sides, half-width) ----
            h2T = {}

            def ffnn(side):
                h1T = act.tile([128, KT_H1, EH], bf, tag=f"h1T{side}", name=f"h1T{side}")
                for mt in range(MT_H1):
                    p = ps.tile([128, EH], f32, tag="mm")
                    for kt in range(KT_F1):
                        nc.tensor.matmul(
                            p[:],
                            w1[side][:, kt, mt * 128 : (mt + 1) * 128],
                            entT[side][:, kt, :],
                            start=(kt == 0),
                            stop=(kt == KT_F1 - 1),
                        )
                    nc.scalar.activation(
                        h1T[:, mt, :],
                        p[:],
                        mybir.ActivationFunctionType.Relu,
                        bias=b1[side][:, mt : mt + 1],
                    )
                h2T[side] = act.tile(
                    [128, KT_H2, EH], bf, tag=f"h2T{side}", name=f"h2T{side}"
                )
                for mt in range(MT_H2):
                    p = ps.tile([128, EH], f32, tag="mm")
                    for kt in range(KT_H1):
                        nc.tensor.matmul(
                            p[:],
                            w2[side][:, kt, mt * 128 : (mt + 1) * 128],
                            h1T[:, kt, :],
                            start=(kt == 0),
                            stop=(kt == KT_H1 - 1),
                        )
                    nc.scalar.activation(
                        h2T[side][:, mt, :],
                        p[:],
                        mybir.ActivationFunctionType.Relu,
                        bias=b2[side][:, mt : mt + 1],
                    )

            ffnn("h")
            ffnn("t")

            # ---- nT[o] = W_bil[o] @ h2h  [128(i), KT_H2, EH(e1)] ----
            nT = []
            for o in range(OUT):
                nTo = act.tile([128, KT_H2, EH], bf, tag=f"nT{o}", name=f"nT{o}")
                for mt in range(MT_H2):
                    p = ps.tile([128, EH], f32, tag="mm")
                    for kt in range(KT_H2):
                        nc.tensor.matmul(
                            p[:],
                            wb[o][:, kt, mt * 128 : (mt + 1) * 128],
                            h2T["h"][:, kt, :],
                            start=(kt == 0),
                            stop=(kt == KT_H2 - 1),
                        )
                    nc.vector.tensor_copy(nTo[:, mt, :], p[:])
                nT.append(nTo)

            # ---- linear terms: linh_o[e1], lint_o[e2] (+ b_lin) ----
            linh, lint = [], []
            for o in range(OUT):
                lh = act.tile([1, EH], bf, tag=f"linh{o}", name=f"linh{o}")
                p = ps1.tile([1, EH], f32, tag="lin")
                for kt in range(KT_H2):
                    nc.tensor.matmul(
                        p[:],
                        wlin[:, kt, o : o + 1],
                        h2T["h"][:, kt, :],
                        start=(kt == 0),
                        stop=(kt == KT_H2 - 1),
                    )
                nc.vector.tensor_copy(lh[:], p[:])
                linh.append(lh)
            for o in range(OUT):
                lt = act.tile([1, EH], bf, tag=f"lint{o}", name=f"lint{o}")
                p = ps1.tile([1, EH], f32, tag="lin")
                for kt in range(KT_H2):
                    nc.tensor.matmul(
                        p[:],
                        wlin[:, KT_H2 + kt, o : o + 1],
                        h2T["t"][:, kt, :],
                        start=(kt == 0),
                        stop=(kt == KT_H2 - 1),
                    )
                nc.scalar.activation(
                    lt[:],
                    p[:],
                    mybir.ActivationFunctionType.Identity,
                    bias=blin[:, o : o + 1],
                )
                lint.append(lt)

            # ---- slab piece [128(e1), OUT, EH(e2)] f32 ----
            out = act.tile([128, OUT, EH], f32, tag="piece")
            for o in range(OUT):
                p = ps.tile([128, EH], f32, tag="mm")
                for kt in range(KT_H2):
                    nc.tensor.matmul(
                        p[:],
                        nT[o][:, kt, :],
                        h2T["t"][:, kt, :],
                        start=(kt == 0),
                        stop=False,
                    )
                nc.tensor.matmul(p[:], linh[o][:], ones[:], start=False, stop=False)
                nc.tensor.matmul(p[:], ones[:], lint[o][:], start=False, stop=True)
                nc.vector.tensor_copy(out[:, o, :], p[:])
            nc.sync.dma_start(
                d_out.ap().rearrange("p (o e) -> p o e", o=OUT), out[:]
            )

    nc.compile()
    _cache["nc"] = nc
    return nc


def _pack(w_arr, kt):
    """[kt*128, n] row-major -> [128, kt*n] partition-packed, bf16."""
    n = w_arr.shape[1]
    return np.ascontiguousarray(
        w_arr.reshape(kt, 128, n).transpose(1, 0, 2).reshape(128, kt * n)
    ).astype(bf16)


def _prep_host(inputs):
    hs = np.asarray(inputs["hidden_states"], dtype=np.float32)
    start = np.asarray(inputs["entity_start"]).astype(np.int64)
    end = np.asarray(inputs["entity_end"]).astype(np.int64)
    label = np.asarray(inputs["entity_label"]).astype(np.int64)

    t = np.arange(T)
    mask = (
        (t[None, None, :] >= start[:, :, None]) & (t[None, None, :] < end[:, :, None])
    ).astype(np.float32)  # [B,E,T]
    counts = np.maximum(mask.sum(-1, keepdims=True), 1.0)
    masknT = (mask / counts).transpose(0, 2, 1)  # [B,T,E]

    def f32(x):
        return np.ascontiguousarray(np.asarray(x, dtype=np.float32))

    emb = f32(inputs["entity_emb_w"])
    Wh1, Wt1 = f32(inputs["Wh1"]), f32(inputs["Wt1"])
    # fold label-embedding half of the ffnn1 contraction into [5, H1] tables
    C1h = emb @ Wh1[D:]  # [5, H1]
    C1t = emb @ Wt1[D:]

    def w1_packed(W1, C1):
        m = np.zeros((128, KT_F1, H1), np.float32)
        m[:, :KT_D, :] = W1[:D].reshape(KT_D, 128, H1).transpose(1, 0, 2)
        m[:NL, KT_D, :] = C1
        return np.ascontiguousarray(m.reshape(128, KT_F1 * H1)).astype(bf16)

    w_bil = f32(inputs["W_bil"])
    shared = {
        "w1h": w1_packed(Wh1, C1h),
        "w1t": w1_packed(Wt1, C1t),
        "w2h": _pack(f32(inputs["Wh2"]), KT_H1),
        "w2t": _pack(f32(inputs["Wt2"]), KT_H1),
        "wb0": _pack(w_bil[0], KT_H2),
        "wb1": _pack(w_bil[1], KT_H2),
        "wlin": _pack(f32(inputs["W_lin"]), 2 * KT_H2),
        "blin": f32(inputs["b_lin"]).reshape(1, OUT),
        "ones": np.ones((1, EH), bf16),
        "bh1t": np.ascontiguousarray(f32(inputs["bh1"]).reshape(MT_H1, 128).T),
        "bt1t": np.ascontiguousarray(f32(inputs["bt1"]).reshape(MT_H1, 128).T),
        "bh2t": np.ascontiguousarray(f32(inputs["bh2"]).reshape(MT_H2, 128).T),
        "bt2t": np.ascontiguousarray(f32(inputs["bt2"]).reshape(MT_H2, 128).T),
    }

    oh5 = np.zeros((B, 128, E), np.float32)
    for b in range(B):
        oh5[b, label[b], np.arange(E)] = 1.0

    in_maps = []
    for i in range(N_CORES):
        b, rest = divmod(i, 4)
        g1, eh = divmod(rest, 2)
        m = dict(shared)
        m["hs"] = _pack(hs[b], KT_T)
        m["mask_h"] = _pack(masknT[b][:, g1 * EH : (g1 + 1) * EH], KT_T)
        m["mask_t"] = _pack(masknT[b][:, eh * EH : (eh + 1) * EH], KT_T)
        m["oh_h"] = np.ascontiguousarray(oh5[b][:, g1 * EH : (g1 + 1) * EH]).astype(bf16)
        m["oh_t"] = np.ascontiguousarray(oh5[b][:, eh * EH : (eh + 1) * EH]).astype(bf16)
        in_maps.append(m)
    return in_maps


def kernel(**inputs) -> np.ndarray:
    in_maps = _prep_host(inputs)
    nc = _build()
    res = run_bass_kernel_spmd(nc, in_maps, list(range(N_CORES)))

    head_idx = np.asarray(inputs["head_idx"]).astype(np.int64)
    tail_idx = np.asarray(inputs["tail_idx"]).astype(np.int64)

    out = np.zeros((B, P, OUT), np.float32)
    for b in range(B):
        slab = np.empty((E, E, OUT), np.float32)
        for rest in range(4):
            g1, eh = divmod(rest, 2)
            piece = res.results[4 * b + rest]["piece"].reshape(128, OUT, EH)
            slab[g1 * EH : (g1 + 1) * EH, eh * EH : (eh + 1) * EH, :] = (
                piece.transpose(0, 2, 1)
            )
        out[b] = slab[head_idx[b], tail_idx[b], :]
    return out
